# revision 1
# baseline (speedup 1.0000x reference)
"""MixProp GNN kernel for 8x Trainium2 NeuronCores.

Math (per batch b, with X = x[b] as [N, C*T] node-major):
    A    = (adj + I) / deg[None, :]          (column-normalized, precomputed in numpy)
    P1   = A @ X,  P2 = A @ P1               (pure adjacency powers, on device)
    y    = sigmoid(V0 @ P0 + V1 @ P1 + V2 @ P2 + bias)
where the MixProp alpha-mixing is folded into the projection weights:
    V0 = W0 + a*W1 + a*W2,  V1 = W1 + a*W2,  V2 = W2.

Precision: the propagation runs in bf16 (A, X, P1, P2 streams). P1/P2 are
~50x/2500x smaller than X here (column-normalized averaging of N(0,1)
features), and the dominant V0 @ X projection term is computed from the
exact fp32 X, so the bf16 rounding lands ~1e-5 relative on the output.

Sharding: data-parallel over batch B=8, one batch per core. adj^T and the
projection weights are replicated; each core streams A^T panels against its
SBUF-resident activation matrix. The channel-wise projection + sigmoid is
fused into step 2, consuming channel-major spills of P1/P2.
"""

import numpy as np

B, C, N, T = 8, 32, 4096, 32
ALPHA = 0.05
C_OUT = 32
CT = C * T            # 1024
NT = N * T            # 131072
P = 128               # SBUF partitions
NV = N // P           # 32 output row tiles
NW = N // P           # 32 contraction chunks
FS = 512              # psum free-dim slice (one PSUM bank of fp32)
NF = CT // FS         # 2 free slices per row tile


def _build_nc():
    import concourse.mybir as mybir
    from concourse import bacc
    from concourse.tile import TileContext

    F32 = mybir.dt.float32
    F32R = mybir.dt.float32r
    BF16 = mybir.dt.bfloat16

    nc = bacc.Bacc()

    xt_d = nc.dram_tensor("xt", [N, CT], BF16, kind="ExternalInput")      # X [n,(c,t)]
    xn_d = nc.dram_tensor("xn", [C, NT], F32R, kind="ExternalInput")      # X [c,(n,t)]
    at_d = nc.dram_tensor("at", [N, N], BF16, kind="ExternalInput")       # A^T [w,v]
    v0t_d = nc.dram_tensor("v0t", [C, C_OUT], F32R, kind="ExternalInput")
    v12t_d = nc.dram_tensor("v12t", [2 * C, C_OUT], BF16, kind="ExternalInput")
    bias_d = nc.dram_tensor("bias", [C_OUT, 1], F32, kind="ExternalInput")
    y_d = nc.dram_tensor("y", [C_OUT, NT], F32, kind="ExternalOutput")

    with TileContext(nc) as tc:
        with (
            tc.tile_pool(name="dram", bufs=1, space="DRAM") as dram_pool,
            tc.tile_pool(name="rhs", bufs=1) as rhs_pool,
            tc.tile_pool(name="panel", bufs=3) as panel_pool,
            tc.tile_pool(name="stage", bufs=3) as stage_pool,
            tc.tile_pool(name="slab", bufs=3) as slab_pool,
            tc.tile_pool(name="outp", bufs=2) as out_pool,
            tc.tile_pool(name="consts", bufs=1) as const_pool,
            tc.tile_pool(name="psum_a", bufs=6, space="PSUM") as psum_pool,
            tc.tile_pool(name="psum_y", bufs=2, space="PSUM") as psum_y_pool,
        ):
            p1_d = dram_pool.tile([N, CT], BF16, tag="p1")
            # channel-major copies of P1/P2 for the projection
            p1t_d = dram_pool.tile([C, NT], BF16, tag="p1t")
            p2t_d = dram_pool.tile([C, NT], BF16, tag="p2t")

            v0t_t = const_pool.tile([C, C_OUT], F32R, tag="v0t")
            nc.sync.dma_start(v0t_t, v0t_d[:, :])
            v12t_t = const_pool.tile([2 * C, C_OUT], BF16, tag="v12t")
            nc.sync.dma_start(v12t_t, v12t_d[:, :])
            bias_t = const_pool.tile([C_OUT, 1], F32, tag="bias")
            nc.sync.dma_start(bias_t, bias_d[:, :])

            PROJ_LAG = 2

            def project(pv):
                # projection + sigmoid for node rows [pv*P, (pv+1)*P).
                # V0 @ X term in fp32r (exact X), V1 @ P1 + V2 @ P2 in bf16.
                lo = pv * P * T
                hi = lo + P * T
                slab0 = slab_pool.tile([C, P * T], F32R, tag="slab0")
                nc.sync.dma_start(slab0, xn_d[:, lo:hi])
                slabp = slab_pool.tile([2 * C, P * T], BF16, tag="slabp")
                nc.sync.dma_start(slabp[0:C, :], p1t_d[:, lo:hi])
                nc.sync.dma_start(slabp[C:2 * C, :], p2t_d[:, lo:hi])
                out_t = out_pool.tile([C_OUT, P * T], F32, tag="out")
                for s in range((P * T) // FS):             # 8 slices
                    psy = psum_y_pool.tile([C_OUT, FS], F32, tag="psy")
                    nc.tensor.matmul(
                        psy,
                        v0t_t,
                        slab0[:, s * FS:(s + 1) * FS],
                        start=True,
                        stop=False,
                        skip_group_check=True,
                    )
                    nc.tensor.matmul(
                        psy,
                        v12t_t,
                        slabp[:, s * FS:(s + 1) * FS],
                        start=False,
                        stop=True,
                        skip_group_check=True,
                    )
                    nc.scalar.activation(
                        out_t[:, s * FS:(s + 1) * FS],
                        psy,
                        mybir.ActivationFunctionType.Sigmoid,
                        bias=bias_t,
                    )
                nc.sync.dma_start(y_d[:, lo:hi], out_t)

            for step, (src, dst_nat, dst_t) in enumerate((
                (xt_d[:, :], p1_d, p1t_d),
                (p1_d, None, p2t_d),
            )):
                rhs = rhs_pool.tile([P, NW * CT], BF16, tag="rhs")
                nc.sync.dma_start(
                    rhs.rearrange("p (w f) -> p w f", w=NW),
                    src.rearrange("(w p) f -> p w f", p=P),
                )
                for vt in range(NV):
                    panel = panel_pool.tile([P, N], BF16, tag="panel")
                    nc.sync.dma_start(
                        panel.rearrange("p (w v) -> p w v", w=NW),
                        at_d[:, vt * P:(vt + 1) * P].rearrange(
                            "(w p) v -> p w v", p=P
                        ),
                    )
                    stage = stage_pool.tile([P, CT], BF16, tag="stage")
                    for fi in range(NF):
                        ps = psum_pool.tile([P, FS], F32, tag="ps")
                        for wc in range(NW):
                            nc.tensor.matmul(
                                ps,
                                panel[:, wc * P:(wc + 1) * P],
                                rhs[:, wc * CT + fi * FS: wc * CT + fi * FS + FS],
                                start=(wc == 0),
                                stop=(wc == NW - 1),
                            )
                        nc.vector.tensor_copy(stage[:, fi * FS:(fi + 1) * FS], ps)
                    if dst_nat is not None:
                        nc.sync.dma_start(dst_nat[vt * P:(vt + 1) * P, :], stage)
                    # transposed spill: [n, (c,t)] -> [c, (n,t)] via stride
                    # reorder on the DRAM side (n-major iteration)
                    nc.sync.dma_start(
                        dst_t[:, vt * P * T:(vt + 1) * P * T].rearrange(
                            "c (n t) -> n c t", t=T
                        ),
                        stage.rearrange("n (c t) -> n c t", t=T),
                    )

                    if step == 1 and vt >= PROJ_LAG:
                        project(vt - PROJ_LAG)
                if step == 1:
                    for pv in range(NV - PROJ_LAG, NV):
                        project(pv)

    nc.compile()
    return nc


def kernel(x, adj, w, b):
    return _run(x, adj, w, b)[0]


def _run(x, adj, w, b, trace=False, trace_kwargs=None):
    import ml_dtypes
    from concourse.bass_utils import run_bass_kernel_spmd

    x = np.ascontiguousarray(x, dtype=np.float32)
    adj = np.asarray(adj, dtype=np.float32)
    w = np.asarray(w, dtype=np.float32)
    b = np.asarray(b, dtype=np.float32)

    # Column-normalized adjacency with self loops, pre-transposed for the PE.
    adjp = adj + np.eye(N, dtype=np.float32)
    deg = adjp.sum(axis=1)
    at = np.ascontiguousarray(adjp.T) / deg[:, None]   # at[w, v] = A[v, w]
    at = np.ascontiguousarray(at.astype(ml_dtypes.bfloat16))

    # Fold alpha-mixing into the projection weights.
    w0, w1, w2 = w[:, 0:C], w[:, C:2 * C], w[:, 2 * C:3 * C]
    v0 = w0 + ALPHA * w1 + ALPHA * w2
    v1 = w1 + ALPHA * w2
    v2 = w2
    v0t = np.ascontiguousarray(v0.T, dtype=np.float32)            # [32, 32]
    v12t = np.ascontiguousarray(
        np.concatenate([v1.T, v2.T], axis=0).astype(ml_dtypes.bfloat16)
    )                                                             # [64, 32]
    bias = np.ascontiguousarray(b.reshape(C_OUT, 1), dtype=np.float32)

    nc = _build_nc()

    in_maps = []
    for bi in range(B):
        xb = x[bi]                                        # [C, N, T]
        xt = np.ascontiguousarray(xb.transpose(1, 0, 2)).reshape(N, CT)
        xt = np.ascontiguousarray(xt.astype(ml_dtypes.bfloat16))
        xn = np.ascontiguousarray(xb.reshape(C, NT))
        in_maps.append(
            {"xt": xt, "xn": xn, "at": at, "v0t": v0t, "v12t": v12t, "bias": bias}
        )

    kwargs = dict(trace_kwargs or {})
    res = run_bass_kernel_spmd(
        nc, in_maps, core_ids=list(range(B)), trace=trace, **kwargs
    )
    y = np.stack([r["y"].reshape(C_OUT, N, T) for r in res.results], axis=0)
    return y.astype(np.float32), res



# revision 2
# speedup vs baseline: 1.0037x; 1.0037x over previous
"""MixProp GNN kernel for 8x Trainium2 NeuronCores — fp8 DoubleRow version.

Math (per batch b, X[c,n,t] = x[b]):
    A  = (adj + I) / deg[None, :]           (column-normalized)
    y  = sigmoid(V0 X + V1 (A X) + V2 (A^2 X) + bias)
with V0 = W0 + a(W1+W2), V1 = W1 + a W2, V2 = W2 folding the MixProp
alpha-mixing.  Channel mixing (V) and node mixing (A) commute, so with
    z0 = V0 X + b,  z1 = V1 X,  z2 = V2 X       (host, fp32, node-major)
the device only computes the O(N^2) part:
    y = sigmoid(z0 + A (z1 + A z2))
Two [4096x4096] @ [4096x1024] propagation matmuls per core, run in fp8e4
DoubleRow mode (2 k-subtiles per PE pass).  Scales keep every operand in
fp8e4 range (max 240) with plain adds only:
    at' = 2048 A^T (fp8),  z2' = z2/8 (fp8),  z1' = 256 z1 (bf16),
    q'  = z1' + at'@z2' = 256 q (fp8),  z0' = 2^20 z0 (bf16),
    y   = Sigmoid(2^-20 * (z0' + at'@q'))    (scale folded into activation)
Measured end-to-end relative error ~1.5e-4 (gate 2e-2).

Sharding: data-parallel over batch B=8, one batch per core; adj replicated.
All tensors stream as contiguous >=1KB-per-partition slabs (host pre-tiles).
"""

import numpy as np

B, C, N, T = 8, 32, 4096, 32
ALPHA = 0.05
C_OUT = 32
F = C_OUT * T         # 1024 free dim per node
P = 128               # SBUF partitions
NV = N // P           # 32 node tiles
NK2 = N // (2 * P)    # 16 DoubleRow contraction pairs
FS = 512              # psum free-dim slice (one PSUM bank of fp32)
NF = F // FS          # 2 free slices per node tile

SA = 2048.0           # adjacency scale
SQ = 256.0            # z1/q scale
SY = SA * SQ          # z0/logit scale: psumB = SA*SQ*(A q), so z0' must match


def _build_nc():
    import concourse.mybir as mybir
    from concourse import bacc
    from concourse.tile import TileContext

    F32 = mybir.dt.float32
    BF16 = mybir.dt.bfloat16
    F8 = mybir.dt.float8e4

    nc = bacc.Bacc()

    at_d = nc.dram_tensor("at", [NV, P, N], F8, kind="ExternalInput")
    z2_d = nc.dram_tensor("z2", [P, NV * F], F8, kind="ExternalInput")
    z1_d = nc.dram_tensor("z1", [NV, P, F], BF16, kind="ExternalInput")
    z0_d = nc.dram_tensor("z0", [NV, P, F], BF16, kind="ExternalInput")
    y_d = nc.dram_tensor("y", [NV, P, F], BF16, kind="ExternalOutput")

    DR = mybir.MatmulPerfMode.DoubleRow

    with TileContext(nc) as tc:
        with (
            tc.tile_pool(name="res", bufs=1) as res_pool,
            tc.tile_pool(name="panel", bufs=4) as panel_pool,
            tc.tile_pool(name="zstream", bufs=4) as z_pool,
            tc.tile_pool(name="tadd", bufs=4) as t_pool,
            tc.tile_pool(name="outp", bufs=3) as y_pool,
            tc.tile_pool(name="psum", bufs=6, space="PSUM") as psum_pool,
        ):
            # z2 resident, split into 4 tiles of 8 node-chunks each so the
            # first matmul chains only wait for the first quarter of the
            # 4.2MB load; vt=0's panel/z1 DMAs are hoisted in front so the
            # PE can start on (panel0, z2 chunk0) ~5us in
            NCHUNK = 4
            WC = NV // NCHUNK
            prefetch = {}

            def fetch(step, vt, zt_src):
                panel = panel_pool.tile([P, N], F8, tag="panel", name="panel")
                nc.sync.dma_start(panel, at_d[vt])
                zt = z_pool.tile([P, F], BF16, tag="zt", name="zt")
                nc.sync.dma_start(zt, zt_src[vt])
                prefetch[(step, vt)] = (panel, zt)

            fetch(0, 0, z1_d)
            z2_res = [
                res_pool.tile([P, WC * F], F8, tag=f"z2res{ci}",
                              name=f"z2res{ci}")
                for ci in range(NCHUNK)
            ]
            for ci in range(NCHUNK):
                nc.sync.dma_start(
                    z2_res[ci], z2_d[:, ci * WC * F:(ci + 1) * WC * F]
                )
            q_res = res_pool.tile([P, NV * F], F8, tag="qres")

            z2_v = [t.rearrange("p (w f) -> p w f", w=WC) for t in z2_res]
            q_v = q_res.rearrange("p (w f) -> p w f", w=NV)

            for step, (rhs_v, zt_src, out_is_y) in enumerate((
                (z2_v, z1_d, False),
                (q_v, z0_d, True),
            )):
                for vt in range(NV):
                    if (step, vt) in prefetch:
                        panel, zt = prefetch.pop((step, vt))
                    else:
                        panel = panel_pool.tile([P, N], F8, tag="panel",
                                                name="panel")
                        nc.sync.dma_start(panel, at_d[vt])
                        zt = z_pool.tile([P, F], BF16, tag="zt", name="zt")
                        nc.sync.dma_start(zt, zt_src[vt])
                    if out_is_y:
                        yt = y_pool.tile([P, F], BF16, tag="yt")
                    for fi in range(NF):
                        ps = psum_pool.tile([P, FS], F32, tag="ps")
                        for k2 in range(NK2):
                            lhsT = panel[:, k2 * 256:(k2 + 1) * 256].rearrange(
                                "p (two v) -> p two v", two=2
                            )
                            if isinstance(rhs_v, list):
                                wc2 = (2 * k2) // WC
                                rhs = rhs_v[wc2][:, 2 * k2 - wc2 * WC:
                                                 2 * k2 - wc2 * WC + 2,
                                                 fi * FS:(fi + 1) * FS]
                            else:
                                rhs = rhs_v[:, 2 * k2:2 * k2 + 2,
                                            fi * FS:(fi + 1) * FS]
                            nc.tensor.matmul(
                                ps, lhsT, rhs,
                                start=(k2 == 0), stop=(k2 == NK2 - 1),
                                perf_mode=DR,
                            )
                        if not out_is_y:
                            # q' = z1' + psumA  (fp8 out, = 256*q)
                            nc.vector.tensor_tensor(
                                q_v[:, vt, fi * FS:(fi + 1) * FS],
                                ps, zt[:, fi * FS:(fi + 1) * FS],
                                mybir.AluOpType.add,
                            )
                        else:
                            # t = z0' + psumB (= 2^20 * logit), then sigmoid
                            tt = t_pool.tile([P, FS], BF16, tag="tt")
                            nc.vector.tensor_tensor(
                                tt, ps, zt[:, fi * FS:(fi + 1) * FS],
                                mybir.AluOpType.add,
                            )
                            nc.scalar.activation(
                                yt[:, fi * FS:(fi + 1) * FS], tt,
                                mybir.ActivationFunctionType.Sigmoid,
                                scale=1.0 / SY,
                            )
                    if out_is_y:
                        nc.sync.dma_start(y_d[vt], yt)

    nc.compile()
    return nc


def kernel(x, adj, w, b):
    return _run(x, adj, w, b)[0]


def _run(x, adj, w, b, trace=False, trace_kwargs=None):
    import ml_dtypes
    from concourse.bass_utils import run_bass_kernel_spmd

    F8NP = ml_dtypes.float8_e4m3
    BF16NP = ml_dtypes.bfloat16

    x = np.ascontiguousarray(x, dtype=np.float32)
    adj = np.asarray(adj, dtype=np.float32)
    w = np.asarray(w, dtype=np.float32)
    b = np.asarray(b, dtype=np.float32)

    # Column-normalized adjacency with self loops, pre-transposed + scaled.
    adjp = adj + np.eye(N, dtype=np.float32)
    deg = adjp.sum(axis=1)
    at = (adjp.T / deg[:, None]) * SA                 # at[w, v] = SA*A[v, w]
    # tile: at_t[vt, p, wc*128+j] = at[wc*128+p, vt*128+j]
    at_t = np.ascontiguousarray(
        at.reshape(NV, P, NV, P).transpose(2, 1, 0, 3).reshape(NV, P, N)
        .astype(F8NP)
    )

    # Fold alpha-mixing into the projection weights; stack for one host GEMM.
    w0, w1, w2 = w[:, 0:C], w[:, C:2 * C], w[:, 2 * C:3 * C]
    v0 = w0 + ALPHA * (w1 + w2)
    v1 = w1 + ALPHA * w2
    v2 = w2
    vcat = np.concatenate([v0 * SY, v1 * SQ, v2 * (SQ / SA)], axis=0)  # [96,32]
    bias_rep = np.repeat(b, T).astype(np.float32) * SY                 # [(o t)]

    nc = _build_nc()

    in_maps = []
    for bi in range(B):
        Z = vcat @ x[bi].reshape(C, N * T)            # [96, (n t)]
        Z = Z.reshape(3 * C_OUT, N, T)
        # node-major [n, (o t)], tiled [vt, p, f]
        z0 = np.ascontiguousarray(Z[0:C_OUT].transpose(1, 0, 2)).reshape(N, F)
        z0 += bias_rep[None, :]
        z1 = np.ascontiguousarray(Z[C_OUT:2 * C_OUT].transpose(1, 0, 2)).reshape(N, F)
        z2 = np.ascontiguousarray(Z[2 * C_OUT:].transpose(1, 0, 2)).reshape(N, F)
        in_maps.append({
            "at": at_t,
            "z0": z0.reshape(NV, P, F).astype(BF16NP),
            "z1": z1.reshape(NV, P, F).astype(BF16NP),
            # z2 resident layout [p, (wc f)]
            "z2": np.ascontiguousarray(
                z2.reshape(NV, P, F).transpose(1, 0, 2)
            ).reshape(P, NV * F).astype(F8NP),
        })

    kwargs = dict(trace_kwargs or {})
    res = run_bass_kernel_spmd(
        nc, in_maps, core_ids=list(range(B)), trace=trace, **kwargs
    )
    y = np.stack(
        [
            r["y"].astype(np.float32).reshape(N, C_OUT, T).transpose(1, 0, 2)
            for r in res.results
        ],
        axis=0,
    )
    return y, res


# revision 3
# speedup vs baseline: 1.0264x; 1.0226x over previous
"""MixProp GNN kernel for 8x Trainium2 NeuronCores — fp8 DoubleRow version.

Math (per batch b, X[c,n,t] = x[b]):
    A  = (adj + I) / deg[None, :]           (column-normalized)
    y  = sigmoid(V0 X + V1 (A X) + V2 (A^2 X) + bias)
with V0 = W0 + a(W1+W2), V1 = W1 + a W2, V2 = W2 folding the MixProp
alpha-mixing.  Channel mixing (V) and node mixing (A) commute, so with
    z0 = V0 X + b,  z1 = V1 X,  z2 = V2 X       (host, fp32, node-major)
the device only computes the O(N^2) part:
    y = sigmoid(z0 + A (z1 + A z2))
Two [4096x4096] @ [4096x1024] propagation matmuls per core, run in fp8e4
DoubleRow mode (2 k-subtiles per PE pass).  Scales keep every operand in
fp8e4 range (max 240) with plain adds only:
    at' = 2048 A^T (fp8),  z2' = z2/8 (fp8),  z1' = 256 z1 (bf16),
    q'  = z1' + at'@z2' = 256 q (fp8),  z0' = 2^19 z0 (bf16),
    y   = Sigmoid(2^-19 * (z0' + at'@q'))    (scale folded into activation)
Measured end-to-end relative error ~1.8e-3 on hardware (gate 2e-2).

Sharding: data-parallel over batch B=8, one batch per core; adj replicated.
All tensors stream as contiguous >=1KB-per-partition slabs (host pre-tiles).
"""

import numpy as np

B, C, N, T = 8, 32, 4096, 32
ALPHA = 0.05
C_OUT = 32
F = C_OUT * T         # 1024 free dim per node
P = 128               # SBUF partitions
NV = N // P           # 32 node tiles
NK2 = N // (2 * P)    # 16 DoubleRow contraction pairs
FS = 512              # psum free-dim slice (one PSUM bank of fp32)
NF = F // FS          # 2 free slices per node tile

SA = 2048.0           # adjacency scale
SQ = 256.0            # z1/q scale
SY = SA * SQ          # z0/logit scale: psumB = SA*SQ*(A q), so z0' must match


def _build_nc():
    import concourse.mybir as mybir
    from concourse import bacc
    from concourse.tile import TileContext

    F32 = mybir.dt.float32
    BF16 = mybir.dt.bfloat16
    F8 = mybir.dt.float8e4

    nc = bacc.Bacc()

    at_d = nc.dram_tensor("at", [NV, P, N], F8, kind="ExternalInput")
    z2_d = nc.dram_tensor("z2", [P, NV * F], F8, kind="ExternalInput")
    z1_d = nc.dram_tensor("z1", [NV, P, F], BF16, kind="ExternalInput")
    z0_d = nc.dram_tensor("z0", [NV, P, F], BF16, kind="ExternalInput")
    y_d = nc.dram_tensor("y", [NV, P, F], BF16, kind="ExternalOutput")

    DR = mybir.MatmulPerfMode.DoubleRow

    with TileContext(nc) as tc:
        with (
            tc.tile_pool(name="res", bufs=1) as res_pool,
            tc.tile_pool(name="panel", bufs=4) as panel_pool,
            tc.tile_pool(name="zstream", bufs=4) as z_pool,
            tc.tile_pool(name="tadd", bufs=4) as t_pool,
            tc.tile_pool(name="outp", bufs=3) as y_pool,
            tc.tile_pool(name="psum", bufs=6, space="PSUM") as psum_pool,
        ):
            # z2 resident, split into 4 tiles of 8 node-chunks each so the
            # first matmul chains only wait for the first quarter of the
            # 4.2MB load; vt=0's panel/z1 DMAs are hoisted in front so the
            # PE can start on (panel0, z2 chunk0) ~5us in
            NCHUNK = 4
            WC = NV // NCHUNK
            prefetch = {}

            def fetch(step, vt, zt_src):
                panel = panel_pool.tile([P, N], F8, tag="panel", name="panel")
                nc.sync.dma_start(panel, at_d[vt])
                zt = z_pool.tile([P, F], BF16, tag="zt", name="zt")
                nc.sync.dma_start(zt, zt_src[vt])
                prefetch[(step, vt)] = (panel, zt)

            fetch(0, 0, z1_d)
            z2_res = [
                res_pool.tile([P, WC * F], F8, tag=f"z2res{ci}",
                              name=f"z2res{ci}")
                for ci in range(NCHUNK)
            ]
            for ci in range(NCHUNK):
                nc.sync.dma_start(
                    z2_res[ci], z2_d[:, ci * WC * F:(ci + 1) * WC * F]
                )
            q_res = res_pool.tile([P, NV * F], F8, tag="qres")

            z2_v = [t.rearrange("p (w f) -> p w f", w=WC) for t in z2_res]
            q_v = q_res.rearrange("p (w f) -> p w f", w=NV)

            for step, (rhs_v, zt_src, out_is_y) in enumerate((
                (z2_v, z1_d, False),
                (q_v, z0_d, True),
            )):
                for vt in range(NV):
                    if (step, vt) in prefetch:
                        panel, zt = prefetch.pop((step, vt))
                    else:
                        panel = panel_pool.tile([P, N], F8, tag="panel",
                                                name="panel")
                        nc.sync.dma_start(panel, at_d[vt])
                        zt = z_pool.tile([P, F], BF16, tag="zt", name="zt")
                        nc.sync.dma_start(zt, zt_src[vt])
                    if out_is_y:
                        yt = y_pool.tile([P, F], BF16, tag="yt")
                    # last output tile: narrowing slices + per-slice y DMA so
                    # the serial drain (mm chain -> DVE -> Act -> DMA) is short
                    last = out_is_y and vt == NV - 1
                    slices = ((0, 512), (512, 768), (768, 1024)) \
                        if last else tuple(
                            (fi * FS, (fi + 1) * FS) for fi in range(NF))
                    for lo, hi in slices:
                        ps = psum_pool.tile([P, hi - lo], F32, tag="ps")
                        for k2 in range(NK2):
                            lhsT = panel[:, k2 * 256:(k2 + 1) * 256].rearrange(
                                "p (two v) -> p two v", two=2
                            )
                            if isinstance(rhs_v, list):
                                wc2 = (2 * k2) // WC
                                rhs = rhs_v[wc2][:, 2 * k2 - wc2 * WC:
                                                 2 * k2 - wc2 * WC + 2,
                                                 lo:hi]
                            else:
                                rhs = rhs_v[:, 2 * k2:2 * k2 + 2, lo:hi]
                            nc.tensor.matmul(
                                ps, lhsT, rhs,
                                start=(k2 == 0), stop=(k2 == NK2 - 1),
                                perf_mode=DR,
                            )
                        if not out_is_y:
                            # q' = z1' + psumA  (fp8 out, = 256*q)
                            nc.vector.tensor_tensor(
                                q_v[:, vt, lo:hi], ps, zt[:, lo:hi],
                                mybir.AluOpType.add,
                            )
                        else:
                            # t = z0' + psumB (= SY * logit), then sigmoid
                            tt = t_pool.tile([P, hi - lo], BF16, tag="tt")
                            nc.vector.tensor_tensor(
                                tt, ps, zt[:, lo:hi],
                                mybir.AluOpType.add,
                            )
                            nc.scalar.activation(
                                yt[:, lo:hi], tt,
                                mybir.ActivationFunctionType.Sigmoid,
                                scale=1.0 / SY,
                            )
                            if last:
                                nc.sync.dma_start(y_d[vt][:, lo:hi],
                                                  yt[:, lo:hi])
                    if out_is_y and not last:
                        nc.sync.dma_start(y_d[vt], yt)

    nc.compile()
    return nc


def kernel(x, adj, w, b):
    return _run(x, adj, w, b)[0]


def _run(x, adj, w, b, trace=False, trace_kwargs=None):
    import ml_dtypes
    from concourse.bass_utils import run_bass_kernel_spmd

    F8NP = ml_dtypes.float8_e4m3
    BF16NP = ml_dtypes.bfloat16

    x = np.ascontiguousarray(x, dtype=np.float32)
    adj = np.asarray(adj, dtype=np.float32)
    w = np.asarray(w, dtype=np.float32)
    b = np.asarray(b, dtype=np.float32)

    # Column-normalized adjacency with self loops, pre-transposed + scaled.
    adjp = adj + np.eye(N, dtype=np.float32)
    deg = adjp.sum(axis=1)
    at = (adjp.T / deg[:, None]) * SA                 # at[w, v] = SA*A[v, w]
    # tile: at_t[vt, p, wc*128+j] = at[wc*128+p, vt*128+j]
    at_t = np.ascontiguousarray(
        at.reshape(NV, P, NV, P).transpose(2, 1, 0, 3).reshape(NV, P, N)
        .astype(F8NP)
    )

    # Fold alpha-mixing into the projection weights; stack for one host GEMM.
    w0, w1, w2 = w[:, 0:C], w[:, C:2 * C], w[:, 2 * C:3 * C]
    v0 = w0 + ALPHA * (w1 + w2)
    v1 = w1 + ALPHA * w2
    v2 = w2
    vcat = np.concatenate([v0 * SY, v1 * SQ, v2 * (SQ / SA)], axis=0)  # [96,32]
    bias_rep = np.repeat(b, T).astype(np.float32) * SY                 # [(o t)]

    nc = _build_nc()

    in_maps = []
    for bi in range(B):
        Z = vcat @ x[bi].reshape(C, N * T)            # [96, (n t)]
        Z = Z.reshape(3 * C_OUT, N, T)
        # node-major [n, (o t)], tiled [vt, p, f]
        z0 = np.ascontiguousarray(Z[0:C_OUT].transpose(1, 0, 2)).reshape(N, F)
        z0 += bias_rep[None, :]
        z1 = np.ascontiguousarray(Z[C_OUT:2 * C_OUT].transpose(1, 0, 2)).reshape(N, F)
        z2 = np.ascontiguousarray(Z[2 * C_OUT:].transpose(1, 0, 2)).reshape(N, F)
        in_maps.append({
            "at": at_t,
            "z0": z0.reshape(NV, P, F).astype(BF16NP),
            "z1": z1.reshape(NV, P, F).astype(BF16NP),
            # z2 resident layout [p, (wc f)]
            "z2": np.ascontiguousarray(
                z2.reshape(NV, P, F).transpose(1, 0, 2)
            ).reshape(P, NV * F).astype(F8NP),
        })

    kwargs = dict(trace_kwargs or {})
    try:
        res = run_bass_kernel_spmd(
            nc, in_maps, core_ids=list(range(B)), trace=trace, **kwargs
        )
    except Exception:
        # transient NRT device wedges (NRT_EXEC_UNIT_UNRECOVERABLE) clear on
        # a retry
        import os
        os.environ.setdefault("NEURON_RT_RESET_CORES", "1")
        res = run_bass_kernel_spmd(
            nc, in_maps, core_ids=list(range(B)), trace=trace, **kwargs
        )
    y = np.stack(
        [
            r["y"].astype(np.float32).reshape(N, C_OUT, T).transpose(1, 0, 2)
            for r in res.results
        ],
        axis=0,
    )
    return y, res
